# revision 11
# baseline (speedup 1.0000x reference)
"""Trainium2 Bass kernel for nn_BHLinear: x -> D0 -> FWHT/64 -> D1 -> FWHT/64 -> final_B.

Math (per row, f = 12-bit feature index = 64*u + v, u = 2m+j):
  FWHT_4096 = H64(u) (x) H64(v).  H64(v) folds into the adjacent block matmuls
  (C1 = H64@B1/64 per-u; G = H-half@final_B/64 per-out-block).  The remaining
  H64(u) = H2(j) (x) H32(m): H2 folds into the block-diag passes, H32 runs as
  dedicated fixed-weight passes (I4 (x) H32) with m brought onto partitions by
  32x32 DVE stream transposes of fp16 pairs viewed as int32.

Five TensorE passes (P1 per-m, P2 fixed I4(x)H32, P3 per-M, P4 fixed, P5 per-W),
each followed by a PSUM drain (fp32->fp16, split ~3:1 scalar:vector) and — for
P1..P4 — a DVE pair transpose.

v2 layout: y tiles store col = rp*64 + blk*2 + rl (rp = row-pair, rl = row
parity, blk = 5-bit feature-block index).  As int32 pairs: col32 = rp*32 + blk,
so the stream transpose runs with contiguous last dims (8.7us vs 15.7us for the
m-major layout).  P1/P3/P5 write PSUM strided ((rp, dm, rl) physical order) so
every drain is a contiguous-read + 16B-run-write copy at full engine rate.
Output is written fp16 (g-major [T,8,128,2048]) and cast to fp32 on host.

Sharding: rows (4*4096=16384) split contiguously across 8 cores (2048 each).
Host packs x into [T=4, 128, 32*R] fp16 (p=(j,v), col=m*R+r).
"""

import numpy as np

# ---- static config ---------------------------------------------------------
NCORES = 8
R = 512                  # rows per tile
T = 4                    # tiles per core
ROWS_PER_CORE = R * T    # 2048
D = 4096
TOTAL_ROWS = NCORES * ROWS_PER_CORE  # 16384

_F16 = np.float16


def _hadamard(n):
    H = np.array([[1.0]], dtype=np.float64)
    while H.shape[0] < n:
        H = np.block([[H, H], [H, -H]])
    return H


_H2 = _hadamard(2)
_H32 = _hadamard(32)
_H64 = _hadamard(64)


def _build_weights(inner_B, final_B):
    """w1/w3/w5 [128,4096] fp16 (32 lhsT blocks side by side), w2 [128,128]."""
    B0 = inner_B[0].astype(np.float64)
    B1 = inner_B[1].astype(np.float64)
    fB = final_B.astype(np.float64)

    C1 = np.einsum('vk,ukt->uvt', _H64, B1) / 64.0
    G = np.zeros((64, 64, 64))
    for u in range(64):
        for h in range(2):
            G[u][:, 32 * h:32 * h + 32] = _H64[:, 32 * h:32 * h + 32] @ fB[2 * u + h] / 64.0

    w1 = np.zeros((128, 32, 128))
    w3 = np.zeros((128, 32, 128))
    w5 = np.zeros((128, 32, 128))
    for m in range(32):
        for j in range(2):
            for jp in range(2):
                w1[j * 64:(j + 1) * 64, m, jp * 64:(jp + 1) * 64] = _H2[j, jp] * B0[2 * m + j]
                w3[j * 64:(j + 1) * 64, m, jp * 64:(jp + 1) * 64] = _H2[j, jp] * C1[2 * m + j]
        for jpp in range(2):
            w5[jpp * 64:(jpp + 1) * 64, m, jpp * 64:(jpp + 1) * 64] = G[2 * m + jpp]
    w1 = w1.reshape(128, 4096)
    w3 = w3.reshape(128, 4096)
    w5 = w5.reshape(128, 4096)
    w2 = np.kron(np.eye(4), _H32)
    return (w1.astype(_F16), w2.astype(_F16), w3.astype(_F16), w5.astype(_F16))


def _pack_x(x):
    """x [..., 4096] fp32 -> list of per-core arrays [T, 128, 32*R] fp16."""
    xf = np.ascontiguousarray(x.reshape(-1, D))
    assert xf.shape[0] == TOTAL_ROWS
    x6 = xf.reshape(NCORES, T, R, 32, 2, 64)       # core,t,r,m,j,v
    x6 = x6.transpose(0, 1, 4, 5, 3, 2)            # core,t,j,v,m,r
    x6 = np.ascontiguousarray(x6).reshape(NCORES, T, 128, 32 * R)
    return [np.ascontiguousarray(x6[c]).astype(_F16) for c in range(NCORES)]


def _unpack_out(outs, orig_shape):
    """outs: list of per-core [T, 8, 128, 2048] fp16 -> [*orig_shape[:-1], 4096] f32.

    out[t][g][p = j3*64 + o64][dm*512 + rp*2 + rl]:
      row = t*512 + rp*2 + rl; m4 = 4g + dm; f = 64*(2*m4 + j3) + o64
    """
    o = np.stack(outs, axis=0).astype(np.float32)            # [c, T, 8, 128, 2048]
    o = o.reshape(NCORES, T, 8, 2, 64, 4, 256, 2)            # c,t,g,j3,o64,dm,rp,rl
    o = o.transpose(0, 1, 6, 7, 2, 5, 3, 4)                  # c,t,rp,rl,g,dm,j3,o64
    o = np.ascontiguousarray(o).reshape(TOTAL_ROWS, D)
    return o.reshape(*orig_shape[:-1], D)


# ---- bass program ----------------------------------------------------------
_PROGRAM = None


def _build_program():
    global _PROGRAM
    if _PROGRAM is not None:
        return _PROGRAM
    from contextlib import ExitStack
    import concourse.tile as tile
    from concourse import bacc, mybir

    f32 = mybir.dt.float32
    f16 = mybir.dt.float16
    i32 = mybir.dt.int32

    nc = bacc.Bacc()
    x_d = nc.declare_dram_parameter("x", [T, 128, 32 * R], f16, isOutput=False)
    w1_d = nc.declare_dram_parameter("w1", [128, 4096], f16, isOutput=False)
    w2_d = nc.declare_dram_parameter("w2", [128, 128], f16, isOutput=False)
    w3_d = nc.declare_dram_parameter("w3", [128, 4096], f16, isOutput=False)
    w5_d = nc.declare_dram_parameter("w5", [128, 4096], f16, isOutput=False)
    out_d = nc.declare_dram_parameter("out", [T, 8, 128, 2048], f16, isOutput=True)

    C = 32 * R          # 16384 cols per tile
    QC = 8 * R          # x-quarter cols = 4096

    with tile.TileContext(nc) as tc, ExitStack() as ctx:
        wpool = ctx.enter_context(tc.tile_pool(name="weights", bufs=1))
        xt_pool = ctx.enter_context(tc.tile_pool(name="xt", bufs=4))
        yraw_pool = ctx.enter_context(tc.tile_pool(name="yraw", bufs=2))
        yt_pool = ctx.enter_context(tc.tile_pool(name="yt", bufs=2))
        out_pool = ctx.enter_context(tc.tile_pool(name="outp", bufs=4))
        psum = ctx.enter_context(tc.tile_pool(name="ps", bufs=2, space="PSUM"))

        w1_sb = wpool.tile([128, 4096], f16)
        w2_sb = wpool.tile([128, 128], f16)
        w3_sb = wpool.tile([128, 4096], f16)
        w5_sb = wpool.tile([128, 4096], f16)
        nc.sync.dma_start(w1_sb[:], w1_d[:])
        nc.sync.dma_start(w2_sb[:], w2_d[:])
        nc.sync.dma_start(w3_sb[:], w3_d[:])
        nc.sync.dma_start(w5_sb[:], w5_d[:])

        sc = nc.scalar.copy
        vc = nc.vector.tensor_copy

        # PE warmup: dummy matmuls (never drained) lift the HAM clock gate to
        # 2.4 GHz while the first x tile is still loading
        ps_w = psum.tile([128, 2048], f32, tag="ps")
        for i in range(16):
            nc.tensor.matmul(
                ps_w[:, (i % 4) * 512:(i % 4 + 1) * 512],
                w1_sb[:, :128],
                w1_sb[:, (i % 8) * 512:(i % 8) * 512 + 512],
                start=True, stop=True,
            )

        def pair_T(dst, src):
            """DVE 32x32 transpose of int32 pairs, contiguous-last-dim layout:
            col32 = rp*32 + blk; swaps partition-low-5 with blk."""
            in_v = src[:].bitcast(i32).rearrange("p (rp m) -> p rp m", m=32)
            out_v = dst[:].bitcast(i32).rearrange("p (rp tl) -> p rp tl", tl=32)
            nc.vector.transpose(out_v, in_v)

        def mm_group(w_sb, src_fn, g, per_block):
            """4 matmuls filling one [128,2048] PSUM tile; returns the tile."""
            ps = psum.tile([128, 2048], f32, tag="ps")
            for i in range(4):
                m = 4 * g + i
                lhsT = w_sb[:, m * 128:(m + 1) * 128] if per_block else w_sb[:]
                nc.tensor.matmul(
                    ps[:, i * 512:(i + 1) * 512],
                    lhsT,
                    src_fn(m),
                    start=True, stop=True,
                )
            return ps

        def emit_stage_blocked(w_sb, jobs):
            """P1/P3: per-block lhsT; drain via strided PSUM read into 16B-run
            y layout (col = rp*64 + m*2 + rl).  jobs = [(src_fn, dst), ...];
            groups of all jobs are interleaved for pipelining."""
            dvs = [dst[:].rearrange("p (rp mm) -> p rp mm", mm=64)
                   for _, dst in jobs]
            for g in range(8):
                pss = [mm_group(w_sb, src_fn, g, True) for src_fn, _ in jobs]
                for j, ps in enumerate(pss):
                    ps_rs = ps[:].rearrange("p (dm rp rl) -> p rp dm rl",
                                            dm=4, rl=2)
                    eng = vc if g == 7 else sc
                    eng(dvs[j][:, :, g * 8:g * 8 + 8], ps_rs)

        def emit_stage_fixed(jobs):
            """P2/P4: fixed w2 lhsT, contiguous rhs/PSUM/drain."""
            for g in range(8):
                pss = [mm_group(w2_sb, lambda m, src=src: src[:, m * 512:(m + 1) * 512],
                                g, False) for src, _ in jobs]
                for j, ps in enumerate(pss):
                    eng = vc if g == 7 else sc
                    eng(jobs[j][1][:, g * 2048:(g + 1) * 2048], ps[:])

        def emit_p5(jobs):
            """jobs = [(y4t, t), ...]"""
            yvs = [y4t[:].rearrange("p (rp mm) -> p rp mm", mm=64)
                   for y4t, _ in jobs]
            for g in range(8):
                pss = [mm_group(w5_sb, lambda m, yv=yv: yv[:, :, m * 2:m * 2 + 2],
                                g, True) for yv in yvs]
                for j, ps in enumerate(pss):
                    out_sb = out_pool.tile([128, 2048], f16, tag="outp")
                    eng = vc if g >= 1 else sc
                    eng(out_sb[:], ps[:])
                    nc.sync.dma_start(out_d[jobs[j][1]][g][:], out_sb[:])

        def load_x(t):
            xq = []
            for q in range(4):
                xt = xt_pool.tile([128, QC], f16, tag="xt")
                nc.sync.dma_start(xt[:], x_d[t][:, q * QC:(q + 1) * QC])
                xq.append(xt)
            return xq

        for tp in range(T // 2):
            ta, tb = 2 * tp, 2 * tp + 1
            xqa = load_x(ta)
            xqb = load_x(tb)
            ya = {}
            yb = {}
            # P1 runs per-tile (xt pool only holds one tile's quarters)
            ya['1'] = yraw_pool.tile([128, C], f16, tag="yraw", name="y1")
            emit_stage_blocked(w1_sb, [
                (lambda m, xq=xqa: xq[m // 8][:, (m % 8) * R:(m % 8 + 1) * R], ya['1']),
            ])
            yb['1'] = yraw_pool.tile([128, C], f16, tag="yraw", name="y1")
            emit_stage_blocked(w1_sb, [
                (lambda m, xq=xqb: xq[m // 8][:, (m % 8) * R:(m % 8 + 1) * R], yb['1']),
            ])
            for y in (ya, yb):
                y['1t'] = yt_pool.tile([128, C], f16, tag="yt", name="y1t")
                pair_T(y['1t'], y['1'])
            for y in (ya, yb):
                y['2'] = yraw_pool.tile([128, C], f16, tag="yraw", name="y2")
            emit_stage_fixed([(ya['1t'], ya['2'])])
            emit_stage_fixed([(yb['1t'], yb['2'])])
            for y in (ya, yb):
                y['2t'] = yt_pool.tile([128, C], f16, tag="yt", name="y2t")
                pair_T(y['2t'], y['2'])
            for y in (ya, yb):
                y['3'] = yraw_pool.tile([128, C], f16, tag="yraw", name="y3")
            yv2a = ya['2t'][:].rearrange("p (rp mm) -> p rp mm", mm=64)
            yv2b = yb['2t'][:].rearrange("p (rp mm) -> p rp mm", mm=64)
            emit_stage_blocked(w3_sb, [
                (lambda m, yv=yv2a: yv[:, :, m * 2:m * 2 + 2], ya['3']),
            ])
            emit_stage_blocked(w3_sb, [
                (lambda m, yv=yv2b: yv[:, :, m * 2:m * 2 + 2], yb['3']),
            ])
            for y in (ya, yb):
                y['3t'] = yt_pool.tile([128, C], f16, tag="yt", name="y3t")
                pair_T(y['3t'], y['3'])
            for y in (ya, yb):
                y['4'] = yraw_pool.tile([128, C], f16, tag="yraw", name="y4")
            emit_stage_fixed([(ya['3t'], ya['4'])])
            emit_stage_fixed([(yb['3t'], yb['4'])])
            for y in (ya, yb):
                y['4t'] = yt_pool.tile([128, C], f16, tag="yt", name="y4t")
                pair_T(y['4t'], y['4'])
            emit_p5([(ya['4t'], ta)])
            emit_p5([(yb['4t'], tb)])

    nc.finalize()
    _PROGRAM = nc
    return nc


_LAST_RESULTS = None


def _make_in_maps(x, inner_B, final_B):
    w1, w2, w3, w5 = _build_weights(np.asarray(inner_B), np.asarray(final_B))
    x_packed = _pack_x(np.asarray(x, dtype=np.float32))
    return [
        {"x": x_packed[c], "w1": w1, "w2": w2, "w3": w3, "w5": w5}
        for c in range(NCORES)
    ]


def kernel(x, inner_B, final_B, _trace=False):
    global _LAST_RESULTS
    from concourse.bass_utils import run_bass_kernel_spmd

    orig_shape = x.shape
    nc = _build_program()
    in_maps = _make_in_maps(x, inner_B, final_B)
    try:
        res = run_bass_kernel_spmd(nc, in_maps, list(range(NCORES)))
    except Exception:
        # transient NRT device errors have been observed; retry once
        res = run_bass_kernel_spmd(nc, in_maps, list(range(NCORES)))
    _LAST_RESULTS = res
    outs = [np.asarray(res.results[c]["out"]) for c in range(NCORES)]
    return _unpack_out(outs, orig_shape).astype(np.float32)


# revision 12
# speedup vs baseline: 1.0393x; 1.0393x over previous
"""Trainium2 Bass kernel for nn_BHLinear: x -> D0 -> FWHT/64 -> D1 -> FWHT/64 -> final_B.

Math (per row, f = 12-bit feature index = 64*u + v, u = 2m+j):
  FWHT_4096 = H64(u) (x) H64(v).  H64(v) folds into the adjacent block matmuls
  (C1 = H64@B1/64 per-u; G = H-half@final_B/64 per-out-block).  The remaining
  H64(u) = H2(j) (x) H32(m): H2 folds into the block-diag passes, H32 runs as
  dedicated fixed-weight passes (I4 (x) H32) with m brought onto partitions by
  32x32 DVE stream transposes of fp16 pairs viewed as int32.

Five TensorE passes (P1 per-m, P2 fixed I4(x)H32, P3 per-M, P4 fixed, P5 per-W),
each followed by a PSUM drain (fp32->fp16, split ~3:1 scalar:vector) and — for
P1..P4 — a DVE pair transpose.

v2 layout: y tiles store col = rp*64 + blk*2 + rl (rp = row-pair, rl = row
parity, blk = 5-bit feature-block index).  As int32 pairs: col32 = rp*32 + blk,
so the stream transpose runs with contiguous last dims (8.7us vs 15.7us for the
m-major layout).  P1/P3/P5 write PSUM strided ((rp, dm, rl) physical order) so
every drain is a contiguous-read + 16B-run-write copy at full engine rate.
Output is written fp16 (g-major [T,8,128,2048]) and cast to fp32 on host.

Sharding: rows (4*4096=16384) split contiguously across 8 cores (2048 each).
Host packs x into [T=4, 128, 32*R] fp16 (p=(j,v), col=m*R+r).
"""

import numpy as np

# ---- static config ---------------------------------------------------------
NCORES = 8
R = 512                  # rows per tile
T = 4                    # tiles per core
ROWS_PER_CORE = R * T    # 2048
D = 4096
TOTAL_ROWS = NCORES * ROWS_PER_CORE  # 16384

_F16 = np.float16


def _hadamard(n):
    H = np.array([[1.0]], dtype=np.float64)
    while H.shape[0] < n:
        H = np.block([[H, H], [H, -H]])
    return H


_H2 = _hadamard(2)
_H32 = _hadamard(32)
_H64 = _hadamard(64)


def _build_weights(inner_B, final_B):
    """w1/w3/w5 [128,4096] fp16 (32 lhsT blocks side by side), w2 [128,128]."""
    B0 = inner_B[0].astype(np.float64)
    B1 = inner_B[1].astype(np.float64)
    fB = final_B.astype(np.float64)

    C1 = np.einsum('vk,ukt->uvt', _H64, B1) / 64.0
    G = np.zeros((64, 64, 64))
    for u in range(64):
        for h in range(2):
            G[u][:, 32 * h:32 * h + 32] = _H64[:, 32 * h:32 * h + 32] @ fB[2 * u + h] / 64.0

    w1 = np.zeros((128, 32, 128))
    w3 = np.zeros((128, 32, 128))
    w5 = np.zeros((128, 32, 128))
    for m in range(32):
        for j in range(2):
            for jp in range(2):
                w1[j * 64:(j + 1) * 64, m, jp * 64:(jp + 1) * 64] = _H2[j, jp] * B0[2 * m + j]
                w3[j * 64:(j + 1) * 64, m, jp * 64:(jp + 1) * 64] = _H2[j, jp] * C1[2 * m + j]
        for jpp in range(2):
            w5[jpp * 64:(jpp + 1) * 64, m, jpp * 64:(jpp + 1) * 64] = G[2 * m + jpp]
    w1 = w1.reshape(128, 4096)
    w3 = w3.reshape(128, 4096)
    w5 = w5.reshape(128, 4096)
    w2 = np.kron(np.eye(4), _H32)
    return (w1.astype(_F16), w2.astype(_F16), w3.astype(_F16), w5.astype(_F16))


def _pack_x(x):
    """x [..., 4096] fp32 -> list of per-core arrays [T, 128, 32*R] fp16."""
    xf = np.ascontiguousarray(x.reshape(-1, D))
    assert xf.shape[0] == TOTAL_ROWS
    x6 = xf.reshape(NCORES, T, R, 32, 2, 64)       # core,t,r,m,j,v
    x6 = x6.transpose(0, 1, 4, 5, 3, 2)            # core,t,j,v,m,r
    x6 = np.ascontiguousarray(x6).reshape(NCORES, T, 128, 32 * R)
    return [np.ascontiguousarray(x6[c]).astype(_F16) for c in range(NCORES)]


def _unpack_out(outs, orig_shape):
    """outs: list of per-core [T, 8, 128, 2048] fp16 -> [*orig_shape[:-1], 4096] f32.

    out[t][g][p = j3*64 + o64][dm*512 + rp*2 + rl]:
      row = t*512 + rp*2 + rl; m4 = 4g + dm; f = 64*(2*m4 + j3) + o64
    """
    o = np.stack(outs, axis=0).astype(np.float32)            # [c, T, 8, 128, 2048]
    o = o.reshape(NCORES, T, 8, 2, 64, 4, 256, 2)            # c,t,g,j3,o64,dm,rp,rl
    o = o.transpose(0, 1, 6, 7, 2, 5, 3, 4)                  # c,t,rp,rl,g,dm,j3,o64
    o = np.ascontiguousarray(o).reshape(TOTAL_ROWS, D)
    return o.reshape(*orig_shape[:-1], D)


# ---- bass program ----------------------------------------------------------
_PROGRAM = None


def _build_program():
    global _PROGRAM
    if _PROGRAM is not None:
        return _PROGRAM
    from contextlib import ExitStack
    import concourse.tile as tile
    from concourse import bacc, mybir

    f32 = mybir.dt.float32
    f16 = mybir.dt.float16
    i32 = mybir.dt.int32

    nc = bacc.Bacc()
    x_d = nc.declare_dram_parameter("x", [T, 128, 32 * R], f16, isOutput=False)
    w1_d = nc.declare_dram_parameter("w1", [128, 4096], f16, isOutput=False)
    w2_d = nc.declare_dram_parameter("w2", [128, 128], f16, isOutput=False)
    w3_d = nc.declare_dram_parameter("w3", [128, 4096], f16, isOutput=False)
    w5_d = nc.declare_dram_parameter("w5", [128, 4096], f16, isOutput=False)
    out_d = nc.declare_dram_parameter("out", [T, 8, 128, 2048], f16, isOutput=True)

    C = 32 * R          # 16384 cols per tile
    QC = 8 * R          # x-quarter cols = 4096

    with tile.TileContext(nc) as tc, ExitStack() as ctx:
        wpool = ctx.enter_context(tc.tile_pool(name="weights", bufs=1))
        xt_pool = ctx.enter_context(tc.tile_pool(name="xt", bufs=4))
        yraw_pool = ctx.enter_context(tc.tile_pool(name="yraw", bufs=2))
        yt_pool = ctx.enter_context(tc.tile_pool(name="yt", bufs=2))
        out_pool = ctx.enter_context(tc.tile_pool(name="outp", bufs=4))
        psum = ctx.enter_context(tc.tile_pool(name="ps", bufs=2, space="PSUM"))

        w1_sb = wpool.tile([128, 4096], f16)
        w2_sb = wpool.tile([128, 128], f16)
        w3_sb = wpool.tile([128, 4096], f16)
        w5_sb = wpool.tile([128, 4096], f16)
        nc.sync.dma_start(w1_sb[:], w1_d[:])
        nc.sync.dma_start(w2_sb[:], w2_d[:])
        nc.sync.dma_start(w3_sb[:], w3_d[:])
        nc.sync.dma_start(w5_sb[:], w5_d[:])

        sc = nc.scalar.copy
        vc = nc.vector.tensor_copy

        # PE warmup: dummy matmuls (never drained) lift the HAM clock gate to
        # 2.4 GHz while the first x tile is still loading
        ps_w = psum.tile([128, 2048], f32, tag="ps")
        for i in range(16):
            nc.tensor.matmul(
                ps_w[:, (i % 4) * 512:(i % 4 + 1) * 512],
                w1_sb[:, :128],
                w1_sb[:, (i % 8) * 512:(i % 8) * 512 + 512],
                start=True, stop=True,
            )

        def pair_T(dst, src):
            """DVE 32x32 transpose of int32 pairs, contiguous-last-dim layout:
            col32 = rp*32 + blk; swaps partition-low-5 with blk."""
            in_v = src[:].bitcast(i32).rearrange("p (rp m) -> p rp m", m=32)
            out_v = dst[:].bitcast(i32).rearrange("p (rp tl) -> p rp tl", tl=32)
            nc.vector.transpose(out_v, in_v)

        def mm_group(w_sb, src_fn, g, per_block):
            """4 matmuls filling one [128,2048] PSUM tile; returns the tile."""
            ps = psum.tile([128, 2048], f32, tag="ps")
            for i in range(4):
                m = 4 * g + i
                lhsT = w_sb[:, m * 128:(m + 1) * 128] if per_block else w_sb[:]
                nc.tensor.matmul(
                    ps[:, i * 512:(i + 1) * 512],
                    lhsT,
                    src_fn(m),
                    start=True, stop=True,
                )
            return ps

        def emit_stage_blocked(w_sb, jobs):
            """P1/P3: per-block lhsT; drain via strided PSUM read into 16B-run
            y layout (col = rp*64 + m*2 + rl).  jobs = [(src_fn, dst), ...];
            groups of all jobs are interleaved for pipelining."""
            dvs = [dst[:].rearrange("p (rp mm) -> p rp mm", mm=64)
                   for _, dst in jobs]
            for g in range(8):
                pss = [mm_group(w_sb, src_fn, g, True) for src_fn, _ in jobs]
                for j, ps in enumerate(pss):
                    ps_rs = ps[:].rearrange("p (dm rp rl) -> p rp dm rl",
                                            dm=4, rl=2)
                    eng = vc if g >= 6 else sc
                    eng(dvs[j][:, :, g * 8:g * 8 + 8], ps_rs)

        def emit_stage_fixed(jobs):
            """P2/P4: fixed w2 lhsT, contiguous rhs/PSUM/drain."""
            for g in range(8):
                pss = [mm_group(w2_sb, lambda m, src=src: src[:, m * 512:(m + 1) * 512],
                                g, False) for src, _ in jobs]
                for j, ps in enumerate(pss):
                    eng = vc if g >= 6 else sc
                    eng(jobs[j][1][:, g * 2048:(g + 1) * 2048], ps[:])

        def emit_p5(jobs):
            """jobs = [(y4t, t), ...]"""
            yvs = [y4t[:].rearrange("p (rp mm) -> p rp mm", mm=64)
                   for y4t, _ in jobs]
            for g in range(8):
                pss = [mm_group(w5_sb, lambda m, yv=yv: yv[:, :, m * 2:m * 2 + 2],
                                g, True) for yv in yvs]
                for j, ps in enumerate(pss):
                    out_sb = out_pool.tile([128, 2048], f16, tag="outp")
                    eng = vc if g >= 4 else sc
                    eng(out_sb[:], ps[:])
                    nc.sync.dma_start(out_d[jobs[j][1]][g][:], out_sb[:])

        def load_x(t):
            xq = []
            for q in range(4):
                xt = xt_pool.tile([128, QC], f16, tag="xt")
                nc.sync.dma_start(xt[:], x_d[t][:, q * QC:(q + 1) * QC])
                xq.append(xt)
            return xq

        for tp in range(T // 2):
            ta, tb = 2 * tp, 2 * tp + 1
            xqa = load_x(ta)
            xqb = load_x(tb)
            ya = {}
            yb = {}
            # P1 runs per-tile (xt pool only holds one tile's quarters)
            ya['1'] = yraw_pool.tile([128, C], f16, tag="yraw", name="y1")
            emit_stage_blocked(w1_sb, [
                (lambda m, xq=xqa: xq[m // 8][:, (m % 8) * R:(m % 8 + 1) * R], ya['1']),
            ])
            yb['1'] = yraw_pool.tile([128, C], f16, tag="yraw", name="y1")
            emit_stage_blocked(w1_sb, [
                (lambda m, xq=xqb: xq[m // 8][:, (m % 8) * R:(m % 8 + 1) * R], yb['1']),
            ])
            for y in (ya, yb):
                y['1t'] = yt_pool.tile([128, C], f16, tag="yt", name="y1t")
                pair_T(y['1t'], y['1'])
            for y in (ya, yb):
                y['2'] = yraw_pool.tile([128, C], f16, tag="yraw", name="y2")
            emit_stage_fixed([(ya['1t'], ya['2'])])
            emit_stage_fixed([(yb['1t'], yb['2'])])
            for y in (ya, yb):
                y['2t'] = yt_pool.tile([128, C], f16, tag="yt", name="y2t")
                pair_T(y['2t'], y['2'])
            for y in (ya, yb):
                y['3'] = yraw_pool.tile([128, C], f16, tag="yraw", name="y3")
            yv2a = ya['2t'][:].rearrange("p (rp mm) -> p rp mm", mm=64)
            yv2b = yb['2t'][:].rearrange("p (rp mm) -> p rp mm", mm=64)
            emit_stage_blocked(w3_sb, [
                (lambda m, yv=yv2a: yv[:, :, m * 2:m * 2 + 2], ya['3']),
            ])
            emit_stage_blocked(w3_sb, [
                (lambda m, yv=yv2b: yv[:, :, m * 2:m * 2 + 2], yb['3']),
            ])
            for y in (ya, yb):
                y['3t'] = yt_pool.tile([128, C], f16, tag="yt", name="y3t")
                pair_T(y['3t'], y['3'])
            for y in (ya, yb):
                y['4'] = yraw_pool.tile([128, C], f16, tag="yraw", name="y4")
            emit_stage_fixed([(ya['3t'], ya['4'])])
            emit_stage_fixed([(yb['3t'], yb['4'])])
            for y in (ya, yb):
                y['4t'] = yt_pool.tile([128, C], f16, tag="yt", name="y4t")
                pair_T(y['4t'], y['4'])
            emit_p5([(ya['4t'], ta)])
            emit_p5([(yb['4t'], tb)])

    nc.finalize()
    _PROGRAM = nc
    return nc


_LAST_RESULTS = None


def _make_in_maps(x, inner_B, final_B):
    w1, w2, w3, w5 = _build_weights(np.asarray(inner_B), np.asarray(final_B))
    x_packed = _pack_x(np.asarray(x, dtype=np.float32))
    return [
        {"x": x_packed[c], "w1": w1, "w2": w2, "w3": w3, "w5": w5}
        for c in range(NCORES)
    ]


def kernel(x, inner_B, final_B, _trace=False):
    global _LAST_RESULTS
    from concourse.bass_utils import run_bass_kernel_spmd

    orig_shape = x.shape
    nc = _build_program()
    in_maps = _make_in_maps(x, inner_B, final_B)
    try:
        res = run_bass_kernel_spmd(nc, in_maps, list(range(NCORES)))
    except Exception:
        # transient NRT device errors have been observed; retry once
        res = run_bass_kernel_spmd(nc, in_maps, list(range(NCORES)))
    _LAST_RESULTS = res
    outs = [np.asarray(res.results[c]["out"]) for c in range(NCORES)]
    return _unpack_out(outs, orig_shape).astype(np.float32)


# revision 13
# speedup vs baseline: 1.0555x; 1.0156x over previous
"""Trainium2 Bass kernel for nn_BHLinear: x -> D0 -> FWHT/64 -> D1 -> FWHT/64 -> final_B.

Math (per row, f = 12-bit feature index = 64*u + v, u = 2m+j):
  FWHT_4096 = H64(u) (x) H64(v).  H64(v) folds into the adjacent block matmuls
  (C1 = H64@B1/64 per-u; G = H-half@final_B/64 per-out-block).  The remaining
  H64(u) = H2(j) (x) H32(m): H2 folds into the block-diag passes, H32 runs as
  dedicated fixed-weight passes (I4 (x) H32) with m brought onto partitions by
  32x32 DVE stream transposes of fp16 pairs viewed as int32.

Five TensorE passes (P1 per-m, P2 fixed I4(x)H32, P3 per-M, P4 fixed, P5 per-W),
each followed by a PSUM drain (fp32->fp16, split ~3:1 scalar:vector) and — for
P1..P4 — a DVE pair transpose.

v2 layout: y tiles store col = rp*64 + blk*2 + rl (rp = row-pair, rl = row
parity, blk = 5-bit feature-block index).  As int32 pairs: col32 = rp*32 + blk,
so the stream transpose runs with contiguous last dims (8.7us vs 15.7us for the
m-major layout).  P1/P3/P5 write PSUM strided ((rp, dm, rl) physical order) so
every drain is a contiguous-read + 16B-run-write copy at full engine rate.
Output is written fp16 (g-major [T,8,128,2048]) and cast to fp32 on host.

Sharding: rows (4*4096=16384) split contiguously across 8 cores (2048 each).
Host packs x into [T=4, 128, 32*R] fp16 (p=(j,v), col=m*R+r).
"""

import numpy as np

# ---- static config ---------------------------------------------------------
NCORES = 8
R = 512                  # rows per tile
T = 4                    # tiles per core
ROWS_PER_CORE = R * T    # 2048
D = 4096
TOTAL_ROWS = NCORES * ROWS_PER_CORE  # 16384

_F16 = np.float16


def _hadamard(n):
    H = np.array([[1.0]], dtype=np.float64)
    while H.shape[0] < n:
        H = np.block([[H, H], [H, -H]])
    return H


_H2 = _hadamard(2)
_H32 = _hadamard(32)
_H64 = _hadamard(64)


def _build_weights(inner_B, final_B):
    """w1/w3/w5 [128,4096] fp16 (32 lhsT blocks side by side), w2 [128,128]."""
    B0 = inner_B[0].astype(np.float64)
    B1 = inner_B[1].astype(np.float64)
    fB = final_B.astype(np.float64)

    C1 = np.einsum('vk,ukt->uvt', _H64, B1) / 64.0
    G = np.zeros((64, 64, 64))
    for u in range(64):
        for h in range(2):
            G[u][:, 32 * h:32 * h + 32] = _H64[:, 32 * h:32 * h + 32] @ fB[2 * u + h] / 64.0

    w1 = np.zeros((128, 32, 128))
    w3 = np.zeros((128, 32, 128))
    w5 = np.zeros((128, 32, 128))
    for m in range(32):
        for j in range(2):
            for jp in range(2):
                w1[j * 64:(j + 1) * 64, m, jp * 64:(jp + 1) * 64] = _H2[j, jp] * B0[2 * m + j]
                w3[j * 64:(j + 1) * 64, m, jp * 64:(jp + 1) * 64] = _H2[j, jp] * C1[2 * m + j]
        for jpp in range(2):
            w5[jpp * 64:(jpp + 1) * 64, m, jpp * 64:(jpp + 1) * 64] = G[2 * m + jpp]
    w1 = w1.reshape(128, 4096)
    w3 = w3.reshape(128, 4096)
    w5 = w5.reshape(128, 4096)
    w2 = np.kron(np.eye(4), _H32)
    return (w1.astype(_F16), w2.astype(_F16), w3.astype(_F16), w5.astype(_F16))


def _pack_x(x):
    """x [..., 4096] fp32 -> list of per-core arrays [T, 128, 32*R] fp16."""
    xf = np.ascontiguousarray(x.reshape(-1, D))
    assert xf.shape[0] == TOTAL_ROWS
    x6 = xf.reshape(NCORES, T, R, 32, 2, 64)       # core,t,r,m,j,v
    x6 = x6.transpose(0, 1, 4, 5, 3, 2)            # core,t,j,v,m,r
    x6 = np.ascontiguousarray(x6).reshape(NCORES, T, 128, 32 * R)
    return [np.ascontiguousarray(x6[c]).astype(_F16) for c in range(NCORES)]


def _unpack_out(outs, orig_shape):
    """outs: list of per-core [T, 8, 128, 2048] fp16 -> [*orig_shape[:-1], 4096] f32.

    out[t][g][p = j3*64 + o64][dm*512 + rp*2 + rl]:
      row = t*512 + rp*2 + rl; m4 = 4g + dm; f = 64*(2*m4 + j3) + o64
    """
    o = np.stack(outs, axis=0).astype(np.float32)            # [c, T, 8, 128, 2048]
    o = o.reshape(NCORES, T, 8, 2, 64, 4, 256, 2)            # c,t,g,j3,o64,dm,rp,rl
    o = o.transpose(0, 1, 6, 7, 2, 5, 3, 4)                  # c,t,rp,rl,g,dm,j3,o64
    o = np.ascontiguousarray(o).reshape(TOTAL_ROWS, D)
    return o.reshape(*orig_shape[:-1], D)


# ---- bass program ----------------------------------------------------------
_PROGRAM = None


def _build_program():
    global _PROGRAM
    if _PROGRAM is not None:
        return _PROGRAM
    from contextlib import ExitStack
    import concourse.tile as tile
    from concourse import bacc, mybir

    f32 = mybir.dt.float32
    f16 = mybir.dt.float16
    i32 = mybir.dt.int32

    nc = bacc.Bacc()
    x_d = nc.declare_dram_parameter("x", [T, 128, 32 * R], f16, isOutput=False)
    w1_d = nc.declare_dram_parameter("w1", [128, 4096], f16, isOutput=False)
    w2_d = nc.declare_dram_parameter("w2", [128, 128], f16, isOutput=False)
    w3_d = nc.declare_dram_parameter("w3", [128, 4096], f16, isOutput=False)
    w5_d = nc.declare_dram_parameter("w5", [128, 4096], f16, isOutput=False)
    out_d = nc.declare_dram_parameter("out", [T, 8, 128, 2048], f16, isOutput=True)

    C = 32 * R          # 16384 cols per tile
    QC = 8 * R          # x-quarter cols = 4096

    with tile.TileContext(nc) as tc, ExitStack() as ctx:
        wpool = ctx.enter_context(tc.tile_pool(name="weights", bufs=1))
        xt_pool = ctx.enter_context(tc.tile_pool(name="xt", bufs=4))
        yraw_pool = ctx.enter_context(tc.tile_pool(name="yraw", bufs=2))
        yt_pool = ctx.enter_context(tc.tile_pool(name="yt", bufs=2))
        out_pool = ctx.enter_context(tc.tile_pool(name="outp", bufs=4))
        psum = ctx.enter_context(tc.tile_pool(name="ps", bufs=2, space="PSUM"))

        w1_sb = wpool.tile([128, 4096], f16)
        w2_sb = wpool.tile([128, 128], f16)
        w3_sb = wpool.tile([128, 4096], f16)
        w5_sb = wpool.tile([128, 4096], f16)
        nc.sync.dma_start(w1_sb[:], w1_d[:])
        nc.sync.dma_start(w2_sb[:], w2_d[:])
        nc.sync.dma_start(w3_sb[:], w3_d[:])
        nc.sync.dma_start(w5_sb[:], w5_d[:])

        sc = nc.scalar.copy
        vc = nc.vector.tensor_copy

        # PE warmup: dummy matmuls (never drained) lift the HAM clock gate to
        # 2.4 GHz while the first x tile is still loading
        ps_w = psum.tile([128, 2048], f32, tag="ps")
        for i in range(16):
            nc.tensor.matmul(
                ps_w[:, (i % 4) * 512:(i % 4 + 1) * 512],
                w1_sb[:, :128],
                w1_sb[:, (i % 8) * 512:(i % 8) * 512 + 512],
                start=True, stop=True,
            )

        def pair_T(dst, src):
            """DVE 32x32 transpose of int32 pairs, contiguous-last-dim layout:
            col32 = rp*32 + blk; swaps partition-low-5 with blk."""
            in_v = src[:].bitcast(i32).rearrange("p (rp m) -> p rp m", m=32)
            out_v = dst[:].bitcast(i32).rearrange("p (rp tl) -> p rp tl", tl=32)
            nc.vector.transpose(out_v, in_v)

        def mm_group(w_sb, src_fn, g, per_block):
            """4 matmuls filling one [128,2048] PSUM tile; returns the tile."""
            ps = psum.tile([128, 2048], f32, tag="ps")
            for i in range(4):
                m = 4 * g + i
                lhsT = w_sb[:, m * 128:(m + 1) * 128] if per_block else w_sb[:]
                nc.tensor.matmul(
                    ps[:, i * 512:(i + 1) * 512],
                    lhsT,
                    src_fn(m),
                    start=True, stop=True,
                )
            return ps

        def emit_stage_blocked(w_sb, jobs):
            """P1/P3: per-block lhsT; drain via strided PSUM read into 16B-run
            y layout (col = rp*64 + m*2 + rl).  jobs = [(src_fn, dst), ...];
            groups of all jobs are interleaved for pipelining."""
            dvs = [dst[:].rearrange("p (rp mm) -> p rp mm", mm=64)
                   for _, dst in jobs]
            for g in range(8):
                pss = [mm_group(w_sb, src_fn, g, True) for src_fn, _ in jobs]
                for j, ps in enumerate(pss):
                    ps_rs = ps[:].rearrange("p (dm rp rl) -> p rp dm rl",
                                            dm=4, rl=2)
                    eng = vc if g >= 6 else sc
                    eng(dvs[j][:, :, g * 8:g * 8 + 8], ps_rs)

        def emit_stage_fixed(jobs):
            """P2/P4: fixed w2 lhsT, contiguous rhs/PSUM/drain."""
            for g in range(8):
                pss = [mm_group(w2_sb, lambda m, src=src: src[:, m * 512:(m + 1) * 512],
                                g, False) for src, _ in jobs]
                for j, ps in enumerate(pss):
                    eng = vc if g >= 6 else sc
                    eng(jobs[j][1][:, g * 2048:(g + 1) * 2048], ps[:])

        def emit_p5(jobs):
            """jobs = [(y4t, t), ...]"""
            yvs = [y4t[:].rearrange("p (rp mm) -> p rp mm", mm=64)
                   for y4t, _ in jobs]
            for g in range(8):
                pss = [mm_group(w5_sb, lambda m, yv=yv: yv[:, :, m * 2:m * 2 + 2],
                                g, True) for yv in yvs]
                for j, ps in enumerate(pss):
                    out_sb = out_pool.tile([128, 2048], f16, tag="outp")
                    eng = vc if g >= 6 else sc
                    eng(out_sb[:], ps[:])
                    nc.sync.dma_start(out_d[jobs[j][1]][g][:], out_sb[:])

        def load_x(t):
            xq = []
            for q in range(4):
                xt = xt_pool.tile([128, QC], f16, tag="xt")
                nc.sync.dma_start(xt[:], x_d[t][:, q * QC:(q + 1) * QC])
                xq.append(xt)
            return xq

        for tp in range(T // 2):
            ta, tb = 2 * tp, 2 * tp + 1
            xqa = load_x(ta)
            xqb = load_x(tb)
            ya = {}
            yb = {}
            # P1 runs per-tile (xt pool only holds one tile's quarters)
            ya['1'] = yraw_pool.tile([128, C], f16, tag="yraw", name="y1")
            emit_stage_blocked(w1_sb, [
                (lambda m, xq=xqa: xq[m // 8][:, (m % 8) * R:(m % 8 + 1) * R], ya['1']),
            ])
            yb['1'] = yraw_pool.tile([128, C], f16, tag="yraw", name="y1")
            emit_stage_blocked(w1_sb, [
                (lambda m, xq=xqb: xq[m // 8][:, (m % 8) * R:(m % 8 + 1) * R], yb['1']),
            ])
            for y in (ya, yb):
                y['1t'] = yt_pool.tile([128, C], f16, tag="yt", name="y1t")
                pair_T(y['1t'], y['1'])
            for y in (ya, yb):
                y['2'] = yraw_pool.tile([128, C], f16, tag="yraw", name="y2")
            emit_stage_fixed([(ya['1t'], ya['2'])])
            emit_stage_fixed([(yb['1t'], yb['2'])])
            for y in (ya, yb):
                y['2t'] = yt_pool.tile([128, C], f16, tag="yt", name="y2t")
                pair_T(y['2t'], y['2'])
            for y in (ya, yb):
                y['3'] = yraw_pool.tile([128, C], f16, tag="yraw", name="y3")
            yv2a = ya['2t'][:].rearrange("p (rp mm) -> p rp mm", mm=64)
            yv2b = yb['2t'][:].rearrange("p (rp mm) -> p rp mm", mm=64)
            emit_stage_blocked(w3_sb, [
                (lambda m, yv=yv2a: yv[:, :, m * 2:m * 2 + 2], ya['3']),
            ])
            emit_stage_blocked(w3_sb, [
                (lambda m, yv=yv2b: yv[:, :, m * 2:m * 2 + 2], yb['3']),
            ])
            for y in (ya, yb):
                y['3t'] = yt_pool.tile([128, C], f16, tag="yt", name="y3t")
                pair_T(y['3t'], y['3'])
            for y in (ya, yb):
                y['4'] = yraw_pool.tile([128, C], f16, tag="yraw", name="y4")
            emit_stage_fixed([(ya['3t'], ya['4'])])
            emit_stage_fixed([(yb['3t'], yb['4'])])
            for y in (ya, yb):
                y['4t'] = yt_pool.tile([128, C], f16, tag="yt", name="y4t")
                pair_T(y['4t'], y['4'])
            emit_p5([(ya['4t'], ta)])
            emit_p5([(yb['4t'], tb)])

    nc.finalize()
    _PROGRAM = nc
    return nc


_LAST_RESULTS = None


def _make_in_maps(x, inner_B, final_B):
    w1, w2, w3, w5 = _build_weights(np.asarray(inner_B), np.asarray(final_B))
    x_packed = _pack_x(np.asarray(x, dtype=np.float32))
    return [
        {"x": x_packed[c], "w1": w1, "w2": w2, "w3": w3, "w5": w5}
        for c in range(NCORES)
    ]


def kernel(x, inner_B, final_B, _trace=False):
    global _LAST_RESULTS
    from concourse.bass_utils import run_bass_kernel_spmd

    orig_shape = x.shape
    nc = _build_program()
    in_maps = _make_in_maps(x, inner_B, final_B)
    try:
        res = run_bass_kernel_spmd(nc, in_maps, list(range(NCORES)))
    except Exception:
        # transient NRT device errors have been observed; retry once
        res = run_bass_kernel_spmd(nc, in_maps, list(range(NCORES)))
    _LAST_RESULTS = res
    outs = [np.asarray(res.results[c]["out"]) for c in range(NCORES)]
    return _unpack_out(outs, orig_shape).astype(np.float32)
